# revision 3
# baseline (speedup 1.0000x reference)
"""Causal self-attention Trainium2 kernel (8-core SPMD), v2.

Sharding: 8 cores = 4 batches x 2 head-groups (tensor parallel over heads).
Each core computes, for its batch b and its 8 heads:
  QKV projection (transposed layouts), causal attention without
  max-subtraction (scores are O(+-10), safe), and a partial output
  projection over its head-group's rows of W_proj.  The host sums the two
  partial outputs per batch (the "all-reduce" of the hint, done host-side).

v2 changes over v1 (312us baseline):
  - all matmul operands bf16 (fp32r at 1cyc/row needs N>=256; bf16 is
    1cyc/row at any N, halves DMA + SBUF, and enables FWL weight loads).
    Verified numerically: max rel err ~4e-3 vs 2e-2 budget.
  - causal trimming: diagonal q-tiles shorten the score/attnV matmul
    moving range and the exp region to valid columns (-20% attn PE rows,
    -15% ACT exp columns).
  - diagonal mask shrunk from [128,2x512] to the single [128,2x128]
    block that straddles the diagonal (-75% gpsimd select work).
  - wqk resident in SBUF (loaded once, not per q-tile): -12MB DMA.
  - interleaved emission: attention(qt) is ACT(exp)-paced, so the PE
    stream interleaves QKV(qt+1) and proj(qt-1) matmul groups between
    attention iterations to keep the PE busy during exp stalls.

Device layouts (per core):
  xt      [P, NCO, T] bf16  this batch's x^T (host pre-transposed)
  Q^T,K^T [f, t] bf16       f = head-major features (head pair / 128-chunk)
  V_ext   [t, 8*65] bf16    per head: 64 V columns + ones column (softmax
                            denominator falls out of the attn@V matmul)
  S^T     [k, q] f32 psum   scores transposed; exp'd to bf16 P^T
  y^T     [f, t] bf16       normalized attention output, feeds W_proj
  out     [T, C] f32        partial projection output (host adds halves)
"""

import numpy as np

import concourse.bass as bass
import concourse.mybir as mybir
import concourse.tile as tile
from concourse import bacc
from concourse.bass_utils import run_bass_kernel_spmd

F32 = mybir.dt.float32
BF16 = mybir.dt.bfloat16
P = 128


def build_nc(T=2048, C=1024, n_loc_heads=8, debug=False, reps=1,
             mm_dt=mybir.dt.bfloat16, fill=True):
    """Build the per-core SPMD program. T must be a multiple of 512."""
    D = 64
    HL = n_loc_heads              # local heads (8)
    FQK = HL * D                  # 512: Q (and K) features per core
    NQT = T // 512                # q-tiles of 512
    NTC = T // P                  # t-chunks of 128
    NCO = C // P                  # contraction chunks (8)
    NM = 2 * FQK // P             # Q+K feature chunks (8)
    NFC = FQK // P                # y^T feature chunks (4)
    NCT = C // 512                # output column tiles (2)
    NGQ = NM + NFC                # QKV filler groups per tile (12)
    NGP = 4 * NCT                 # proj filler groups per tile (8)
    Exp = mybir.ActivationFunctionType.Exp
    MDT = mm_dt

    nc = bacc.Bacc(target_bir_lowering=False, debug=debug)
    xt = nc.dram_tensor("xt", [P, NCO, T], MDT, kind="ExternalInput")
    wqk = nc.dram_tensor("wqk", [P, NM, NCO, P], MDT, kind="ExternalInput")
    wv = nc.dram_tensor("wv", [P, NCO, FQK], MDT, kind="ExternalInput")
    wpr = nc.dram_tensor("wpr", [P, NFC, C], MDT, kind="ExternalInput")
    bqk = nc.dram_tensor("bqk", [P, NM], F32, kind="ExternalInput")
    bv = nc.dram_tensor("bv", [P, FQK], F32, kind="ExternalInput")
    bpr = nc.dram_tensor("bpr", [P, C], F32, kind="ExternalInput")
    out = nc.dram_tensor("out", [T, C], F32, kind="ExternalOutput")

    with tile.TileContext(nc) as tc:
        with (
            tc.tile_pool(name="const", bufs=1) as cpool,
            tc.tile_pool(name="persist", bufs=1) as ppool,
            tc.tile_pool(name="xt", bufs=2) as xtp,
            tc.tile_pool(name="qt", bufs=2) as qtp,
            tc.tile_pool(name="yt", bufs=2) as ytp,
            tc.tile_pool(name="pt", bufs=3) as ptp,
            tc.tile_pool(name="yx", bufs=2) as yxp,
            tc.tile_pool(name="oout", bufs=2) as outp,
            tc.tile_pool(name="dnm", bufs=2) as dnp,
            tc.tile_pool(name="mm", bufs=2, space="PSUM") as mmp,
            tc.tile_pool(name="sp", bufs=2, space="PSUM") as spp,
            tc.tile_pool(name="yps", bufs=2, space="PSUM") as ypp,
        ):
            # ---- constants / persistent weights (one-time loads) ----
            ones_sb = cpool.tile([P, HL, 1], F32, tag="ones")
            nc.vector.memset(ones_sb[:], 1.0)
            bqk_sb = cpool.tile([P, NM], F32, tag="bqk")
            nc.sync.dma_start(bqk_sb[:], bqk[:, :])
            bv_sb = cpool.tile([P, FQK], F32, tag="bv")
            bpr_sb = cpool.tile([P, C], F32, tag="bpr")

            KT = ppool.tile([P, NFC, T], MDT, tag="KT")
            VE = ppool.tile([P, NTC, HL * (D + 1)], MDT, tag="VE")
            wqk_sb = ppool.tile([P, NM, NCO, P], MDT, tag="wqk")
            wv_sb = ppool.tile([P, NCO, FQK], MDT, tag="wv")
            wpr_sb = ppool.tile([P, NFC, C], MDT, tag="wpr")

            # chunked so the first QKV group only waits on its own m-slice
            for m in range(NM):
                nc.sync.dma_start(wqk_sb[:, m], wqk[:, m])
            nc.sync.dma_start(wv_sb[:], wv[:, :])
            nc.sync.dma_start(bv_sb[:], bv[:, :])
            nc.sync.dma_start(wpr_sb[:], wpr[:, :])
            nc.sync.dma_start(bpr_sb[:], bpr[:, :])

            # ---- per-tile state (rotating) ----
            # tiles are keyed by a monotone counter; state[i] holds the live
            # SBUF tiles for logical tile i (rep*NQT + qt)
            n_tiles = reps * NQT
            xts = {}      # i -> xTt tile
            qts = {}      # i -> QTt tile
            yts = {}      # i -> yTt tile

            def emit_xt_dma(i):
                qt_i = i % NQT
                q0 = qt_i * 512
                xTt = xtp.tile([P, NCO, 512], MDT, tag="xT", name=f"xT{i}")
                nc.sync.dma_start(xTt[:], xt[:, :, q0:q0 + 512])
                xts[i] = xTt

            def emit_qkv_group(i, m):
                qt_i = i % NQT
                q0 = qt_i * 512
                xTt = xts[i]
                if m < NM:
                    if m == 0:
                        qts[i] = qtp.tile([P, NFC, 512], MDT, tag="QTt", name=f"QTt{i}")
                    ps = mmp.tile([P, 512], F32, tag="mm")
                    for co in range(NCO):
                        nc.tensor.matmul(ps[:], wqk_sb[:, m, co, :],
                                         xTt[:, co, :],
                                         start=(co == 0), stop=(co == NCO - 1))
                    if m < NFC:
                        dst = qts[i][:, m, :]
                    else:
                        dst = KT[:, m - NFC, q0:q0 + 512]
                    nc.vector.tensor_scalar_add(dst, ps[:],
                                                bqk_sb[:, m:m + 1])
                else:
                    tc_i = m - NM
                    ps = mmp.tile([P, 512], F32, tag="mm")
                    for co in range(NCO):
                        nc.tensor.matmul(
                            ps[:], xTt[:, co, tc_i * P:(tc_i + 1) * P],
                            wv_sb[:, co, :],
                            start=(co == 0), stop=(co == NCO - 1))
                    tci = qt_i * 4 + tc_i
                    vev = VE[:, tci, :].rearrange("p (h e) -> p h e", e=D + 1)
                    nc.vector.tensor_add(
                        vev[:, :, :D],
                        ps[:].rearrange("p (h d) -> p h d", d=D),
                        bv_sb[:].rearrange("p (h d) -> p h d", d=D))
                    nc.vector.tensor_copy(vev[:, :, D:D + 1], ones_sb[:])

            def emit_proj_group(i, g):
                qt_i = i % NQT
                q0 = qt_i * 512
                tc_i, ct = divmod(g, NCT)
                yTt = yts[i]
                ps = mmp.tile([P, 512], F32, tag="mm")
                for fc in range(NFC):
                    nc.tensor.matmul(
                        ps[:], yTt[:, fc, tc_i * P:(tc_i + 1) * P],
                        wpr_sb[:, fc, ct * 512:(ct + 1) * 512],
                        start=(fc == 0), stop=(fc == NFC - 1))
                ot = outp.tile([P, 512], F32, tag="oout")
                nc.vector.tensor_add(ot[:], ps[:],
                                     bpr_sb[:, ct * 512:(ct + 1) * 512])
                nc.sync.dma_start(
                    out[q0 + tc_i * P:q0 + (tc_i + 1) * P,
                        ct * 512:(ct + 1) * 512], ot[:])
                if g == NGP - 1:
                    del yts[i]

            def emit_attn_iter(i, ch, kc, ypsA, ypsB):
                qt_i = i % NQT
                nk = 4 * (qt_i + 1)
                kcl = kc - 4 * qt_i       # >=0 on the diagonal q-tile band
                qlo = P * kcl if kcl > 0 else 0
                QTt = qts[i]
                sp2 = spp.tile([P, 1024], F32, tag="sp")
                nc.tensor.matmul(
                    sp2[:, qlo:512],
                    KT[0:64, ch, kc * P:(kc + 1) * P],
                    QTt[0:64, ch, qlo:512],
                    start=True, stop=True, tile_position=(0, 0))
                nc.tensor.matmul(
                    sp2[:, 512 + qlo:1024],
                    KT[64:128, ch, kc * P:(kc + 1) * P],
                    QTt[64:128, ch, qlo:512],
                    start=True, stop=True, tile_position=(64, 0))
                pt_t = ptp.tile([P, 1024], MDT, tag="pt")
                sv = sp2[:].rearrange("p (h q) -> p h q", h=2)
                pv = pt_t[:].rearrange("p (h q) -> p h q", h=2)
                nc.scalar.activation(pv[:, :, qlo:512], sv[:, :, qlo:512],
                                     Exp, scale=0.125)
                if kcl >= 0:
                    # zero the invalid (k > q) triangle: it lies entirely in
                    # the single 128-wide block straddling the diagonal
                    nc.gpsimd.affine_select(
                        out=pv[:, :, qlo:qlo + P],
                        in_=pv[:, :, qlo:qlo + P],
                        compare_op=mybir.AluOpType.is_ge,
                        fill=0.0, base=0, channel_multiplier=-1,
                        pattern=[[0, 2], [1, P]])
                hA, hB = 2 * ch, 2 * ch + 1
                nc.tensor.matmul(
                    ypsA[:D + 1, qlo:512],
                    VE[:, kc, hA * (D + 1):(hA + 1) * (D + 1)],
                    pt_t[:, qlo:512],
                    start=(kc == 0), stop=(kc == nk - 1))
                nc.tensor.matmul(
                    ypsB[:D + 1, qlo:512],
                    VE[:, kc, hB * (D + 1):(hB + 1) * (D + 1)],
                    pt_t[:, 512 + qlo:1024],
                    start=(kc == 0), stop=(kc == nk - 1))

            def emit_attn_tail(i, ch, ypsA, ypsB):
                yTt = yts[i]
                for po, yps in ((0, ypsA), (64, ypsB)):
                    yext = yxp.tile([D + 1, 512], F32, tag="yext")
                    nc.vector.tensor_copy(yext[:], yps[:D + 1, :])
                    rd = dnp.tile([1, 512], F32, tag="rd")
                    nc.vector.reciprocal(rd[:], yext[D:D + 1, :])
                    repb = dnp.tile([64, 512], F32, tag="rep")
                    nc.gpsimd.partition_broadcast(repb[:], rd[:])
                    nc.vector.tensor_mul(
                        yTt[po:po + 64, ch, :], yext[:D, :], repb[:])

            # ---- main schedule ----
            filler = []

            def drain(n):
                for _ in range(min(n, len(filler))):
                    filler.pop(0)()

            emit_xt_dma(0)
            for m in range(NGQ):
                emit_qkv_group(0, m)

            for i in range(n_tiles):
                qt_i = i % NQT
                nk = 4 * (qt_i + 1)
                if i + 1 < n_tiles:
                    emit_xt_dma(i + 1)
                    filler.extend(
                        (lambda j=i + 1, m=m: emit_qkv_group(j, m))
                        for m in range(NGQ))
                yts[i] = ytp.tile([P, NFC, 512], MDT, tag="yTt", name=f"yTt{i}")
                n_iters = 4 * nk
                nf = len(filler)
                # spread fillers evenly over this tile's attention iterations
                it = 0
                done = 0
                for ch in range(NFC):
                    ypsA = ypp.tile([P, 512], F32, tag="yps")
                    ypsB = ypp.tile([P, 512], F32, tag="yps")
                    for kc in range(nk):
                        emit_attn_iter(i, ch, kc, ypsA, ypsB)
                        it += 1
                        want = nf * it // n_iters
                        if want > done:
                            drain(want - done)
                            done = want
                    emit_attn_tail(i, ch, ypsA, ypsB)
                drain(len(filler) if i + 1 >= n_tiles else 0)
                filler.extend(
                    (lambda j=i, g=g: emit_proj_group(j, g))
                    for g in range(NGP))
            drain(len(filler))

    nc.compile()
    return nc


_CACHE = {}


def _get_nc():
    if "nc" not in _CACHE:
        _CACHE["nc"] = build_nc()
    return _CACHE["nc"]


def make_in_maps(x, W_attn, b_attn, W_proj, b_proj, B=4, C=1024):
    import ml_dtypes
    bfnp = ml_dtypes.bfloat16
    x = np.ascontiguousarray(np.asarray(x, dtype=np.float32))
    W_attn = np.asarray(W_attn, dtype=np.float32)
    b_attn = np.asarray(b_attn, dtype=np.float32)
    W_proj = np.asarray(W_proj, dtype=np.float32)
    b_proj = np.asarray(b_proj, dtype=np.float32)
    in_maps = []
    for core in range(2 * B):
        b, hg = core // 2, core % 2
        s = slice(hg * 512, (hg + 1) * 512)
        wqk_flat = np.concatenate(
            [W_attn[:, s], W_attn[:, C + hg * 512:C + (hg + 1) * 512]],
            axis=1)  # [C, 1024]
        # device layout [ci, m, co, f]: wqk_flat[co*128+ci, m*128+f]
        wqk_c = np.ascontiguousarray(
            wqk_flat.reshape(8, 128, 8, 128).transpose(1, 2, 0, 3)
            .astype(bfnp))
        # wv [ci, co, n]
        wv_c = np.ascontiguousarray(
            W_attn[:, 2 * C + hg * 512:2 * C + (hg + 1) * 512]
            .reshape(8, 128, 512).transpose(1, 0, 2).astype(bfnp))
        # wpr [fi, fo, n]
        wpr_c = np.ascontiguousarray(
            W_proj[hg * 512:(hg + 1) * 512, :]
            .reshape(4, 128, C).transpose(1, 0, 2).astype(bfnp))
        bqk_vec = np.concatenate([b_attn[s], b_attn[C + hg * 512:
                                                    C + (hg + 1) * 512]])
        bqk_c = np.ascontiguousarray(bqk_vec.reshape(8, 128).T)
        bv_c = np.ascontiguousarray(
            np.tile(b_attn[2 * C + hg * 512:2 * C + (hg + 1) * 512][None, :],
                    (128, 1)))
        if hg == 0:
            bpr_c = np.ascontiguousarray(np.tile(b_proj[None, :], (128, 1)))
        else:
            bpr_c = np.zeros((128, C), dtype=np.float32)
        # xt [ci, co, T]
        xt_c = np.ascontiguousarray(
            x[b].T.reshape(8, 128, 2048).transpose(1, 0, 2).astype(bfnp))
        in_maps.append({
            "xt": xt_c,
            "wqk": wqk_c, "wv": wv_c, "wpr": wpr_c,
            "bqk": bqk_c, "bv": bv_c, "bpr": bpr_c,
        })
    return in_maps


def kernel(x, W_attn, b_attn, W_proj, b_proj):
    B, T, C = 4, 2048, 1024
    nc = _get_nc()
    in_maps = make_in_maps(x, W_attn, b_attn, W_proj, b_proj, B=B, C=C)
    res = run_bass_kernel_spmd(nc, in_maps, list(range(2 * B)))
    out = np.empty((B, T, C), dtype=np.float32)
    for b in range(B):
        out[b] = res.results[2 * b]["out"] + res.results[2 * b + 1]["out"]
    return out
